# revision 1
# baseline (speedup 1.0000x reference)
"""DETR scene-graph predicate head on 8 Trainium2 NeuronCores.

Math: logits[l,b,r,:] = concat(hs[l,b,q_sub], hs[l,b,q_obj]) @ W_pred.T + b_pred
where q_sub/q_obj are derived from (tgt_perm inverse, relationships,
src_indices) — pure integer index math, done on host.

Strategy (batch axis sharded 8 ways; L*B/8 = 192 (layer,image) blocks/core):
  - Host builds, per block, a [101, 384] bf16 row: hs_block [101, 256] next to
    a one-hot selector [101, 128] (col j selects query q_sub[j], j<64, or
    q_obj[j-64]). Blocks are packed in groups of G=8 into one padded
    [128, G*384] DMA (128 partitions keeps all 16 SDMA engines engaged;
    SWDGE/gpsimd queue — the HWDGE path runs at single-engine rate here).
  - Phase A (gather+transpose fused): pAB = hs_chunk.T @ onehot gives the
    gathered pair representation already d-on-partitions, two matmuls per
    block into one shared psum bank per block-pair, one DVE/ACT cast copy
    to bf16 (alternating engines).
  - Phase B: logits[r, p] accumulates 4 matmuls (2 d-chunks x sub/obj)
    against W_pred.T chunks; blocks 2k/2k+1 run concurrently in the left/
    right PE array halves via tile_position (0,0)/(0,64), outputs stacked on
    psum partitions 0:64/64:128 of one group-wide bank.
  - One bias add (DVE) + one store (scalar-queue DMA) per group; host
    unpacks the [NG, 128, 4*51] layout back to [L, B, R, P].
  - A ~4.5us dense-matmul preamble warms the PE clock (HAM) to 2.4 GHz.

hs and W_pred are bf16 on-chip (one-hot gather is exact in bf16; psum
accumulates f32), giving ~2.4e-3 relative error vs the f32 reference.
"""

import sys

import numpy as np

L, B, Q1, D = 6, 256, 101, 256
M, R, P = 64, 64, 51
NCORES = 8
BLOC = B // NCORES          # images per core
NB = L * BLOC               # (layer, image) blocks per core
PK = D + 2 * R              # packed row width: 256 hs + 128 onehot
G = 8                       # blocks per DMA group
NG = NB // G                # groups per core

_CACHE = {}


def _build_program():
    import concourse.bacc as bacc
    import concourse.mybir as mybir
    import concourse.tile as tile
    from contextlib import ExitStack

    f32 = mybir.dt.float32
    bf16 = mybir.dt.bfloat16
    nc = bacc.Bacc("TRN2", target_bir_lowering=False, debug=False)

    # phase-B col-packing: blocks 2k / 2k+1 share the PE array via
    # tile_position (0,0)/(0,64); outputs land on psum partitions 0:64 /
    # 64:128 at column slot k -> group output is [128, (G//2)*P].
    GH = G // 2
    pk = nc.dram_tensor("pk", [NG, 128, G * PK], bf16, kind="ExternalInput").ap()
    wt = nc.dram_tensor("wt", [128, 4 * P], bf16, kind="ExternalInput").ap()
    bias = nc.dram_tensor("bias", [128, GH * P], f32, kind="ExternalInput").ap()
    out = nc.dram_tensor("out", [NG, 128, GH * P], f32, kind="ExternalOutput").ap()

    with tile.TileContext(nc) as tc, ExitStack() as ctx:
        const = ctx.enter_context(tc.tile_pool(name="const", bufs=1))
        inp = ctx.enter_context(tc.tile_pool(name="inp", bufs=5))
        rep = ctx.enter_context(tc.tile_pool(name="rep", bufs=6))
        outp = ctx.enter_context(tc.tile_pool(name="outp", bufs=3))
        psA = ctx.enter_context(tc.tile_pool(name="psA", bufs=4, space="PSUM"))
        psO = ctx.enter_context(tc.tile_pool(name="psO", bufs=2, space="PSUM"))

        wt_t = const.tile([128, 4 * P], bf16)
        nc.sync.dma_start(out=wt_t[:], in_=wt[:])
        bias_t = const.tile([128, GH * P], f32)
        nc.sync.dma_start(out=bias_t[:], in_=bias[:])

        # HAM warm-up: dense N=512 matmuls push the PE clock 1.2 -> 2.4 GHz
        wu = const.tile([128, 512], bf16)
        nc.vector.memset(wu[:], 0.0)
        wps = psA.tile([128, 512], f32, tag="pAB")
        for _ in range(20):
            nc.tensor.matmul(out=wps[:], lhsT=wu[:, 0:128], rhs=wu[:],
                             start=True, stop=True)


        for g in range(NG):
            # one contiguous load per group of G blocks (bf16)
            pk_t = inp.tile([128, G * PK], bf16, tag="pk")
            nc.gpsimd.dma_start(out=pk_t[:], in_=pk[g])
            o_t = outp.tile([128, GH * P], f32, tag="o")
            # all G blocks' phase-B outputs share one psum bank tile
            pO = psO.tile([128, GH * P], f32, tag="pO")

            # pairs of blocks (2k, 2k+1) flow together: 4 gather matmuls into
            # one full psum bank, one cast copy, then 8 col-packed predicate
            # matmuls (left/right array halves run concurrently).
            for k in range(GH):
                j0, j1 = 2 * k, 2 * k + 1
                pAB = psA.tile([128, 512], f32, tag="pAB")
                for s, j in enumerate((j0, j1)):
                    hs_t = pk_t[0:Q1, j * PK:j * PK + D]
                    oh_t = pk_t[0:Q1, j * PK + D:(j + 1) * PK]
                    # pAB cols [s*256 : s*256+256]: [d-chunk0 | d-chunk1],
                    # each [sub 64 | obj 64]
                    nc.tensor.matmul(out=pAB[:, s * 256:s * 256 + 2 * R],
                                     lhsT=hs_t[:, 0:128], rhs=oh_t[:],
                                     start=True, stop=True)
                    nc.tensor.matmul(out=pAB[:, s * 256 + 2 * R:s * 256 + 4 * R],
                                     lhsT=hs_t[:, 128:256], rhs=oh_t[:],
                                     start=True, stop=True)
                bAB = rep.tile([128, 512], bf16, tag="bAB")
                if k % 2 == 0:
                    nc.vector.tensor_copy(out=bAB[:], in_=pAB[:])
                else:
                    nc.scalar.copy(out=bAB[:], in_=pAB[:])

                o0 = pO[0:R, k * P:(k + 1) * P]
                o1 = pO[R:2 * R, k * P:(k + 1) * P]
                for c, (lo, hi) in enumerate(
                        [(0, R), (2 * R, 3 * R), (R, 2 * R), (3 * R, 4 * R)]):
                    wch = wt_t[:, c * P:(c + 1) * P]
                    nc.tensor.matmul(out=o0, lhsT=bAB[:, lo:hi], rhs=wch,
                                     start=(c == 0), stop=(c == 3),
                                     tile_position=(0, 0))
                    nc.tensor.matmul(out=o1, lhsT=bAB[:, 256 + lo:256 + hi],
                                     rhs=wch,
                                     start=(c == 0), stop=(c == 3),
                                     tile_position=(0, 64))

            # one bias add for the whole group, one store per group
            nc.vector.tensor_add(out=o_t[:], in0=pO[:], in1=bias_t[:])
            nc.scalar.dma_start(out=out[g], in_=o_t[:])

    nc.compile()
    return nc


def _host_indices(src_indices, tgt_perm, relationships):
    """q_sub, q_obj: [L, B, R] int64 — matched query slot per relation."""
    src = np.asarray(src_indices, dtype=np.int64)
    tgt = np.asarray(tgt_perm, dtype=np.int64)
    rel = np.asarray(relationships, dtype=np.int64)

    # lookup[l, b, tgt[l, b, k]] = k
    lookup = np.empty((L, B, M), dtype=np.int64)
    li = np.arange(L)[:, None, None]
    bi = np.arange(B)[None, :, None]
    lookup[li, bi, tgt] = np.broadcast_to(np.arange(M), (L, B, M))

    sub_t = np.broadcast_to(rel[None, :, :, 0], (L, B, R))
    obj_t = np.broadcast_to(rel[None, :, :, 1], (L, B, R))
    pos_sub = np.take_along_axis(lookup, sub_t, axis=2)
    pos_obj = np.take_along_axis(lookup, obj_t, axis=2)
    q_sub = np.take_along_axis(src, pos_sub, axis=2)
    q_obj = np.take_along_axis(src, pos_obj, axis=2)
    return q_sub, q_obj


def _host_prepare(hs, src_indices, tgt_perm, relationships, W_pred, b_pred):
    """Build per-core input maps."""
    hs = np.asarray(hs, dtype=np.float32)
    W = np.asarray(W_pred, dtype=np.float32)
    b = np.asarray(b_pred, dtype=np.float32)

    q_sub, q_obj = _host_indices(src_indices, tgt_perm, relationships)
    q_cat = np.concatenate([q_sub, q_obj], axis=-1)          # [L, B, 2R]
    onehot = (np.arange(Q1)[None, None, :, None] == q_cat[:, :, None, :])
    onehot = onehot.astype(np.float32)                        # [L, B, Q1, 2R]

    import ml_dtypes
    bf16 = ml_dtypes.bfloat16

    packed = np.zeros((L, B, 128, PK), dtype=bf16)
    packed[:, :, :Q1, :D] = hs.astype(bf16)
    packed[:, :, :Q1, D:] = onehot

    # W_pred [P, 2D] -> Wt [2D, P] -> packed [128, 4*P] chunk-major
    wt_packed = np.ascontiguousarray(
        W.T.reshape(4, 128, P).transpose(1, 0, 2).reshape(128, 4 * P)
    ).astype(bf16)
    bias_b = np.ascontiguousarray(np.tile(b[None, :], (128, G // 2)))  # [128, GH*P]

    in_maps = []
    for c in range(NCORES):
        sl = slice(c * BLOC, (c + 1) * BLOC)
        pk_core = packed[:, sl].reshape(NB, 128, PK)
        # group-major layout: [NG, Q1, G*PK], block j of group at cols j*PK
        pk_core = np.ascontiguousarray(
            pk_core.reshape(NG, G, 128, PK).transpose(0, 2, 1, 3)
            .reshape(NG, 128, G * PK))
        in_maps.append({
            "pk": pk_core,
            "wt": wt_packed,
            "bias": bias_b,
        })
    return in_maps


def kernel(hs, src_indices, tgt_perm, relationships, W_pred, b_pred):
    if "concourse" not in sys.modules:
        try:
            import concourse  # noqa: F401
        except ImportError:
            sys.path.insert(0, "/opt/trn_rl_repo")
    from concourse import bass_utils

    in_maps = _host_prepare(hs, src_indices, tgt_perm, relationships,
                            W_pred, b_pred)
    if "nc" not in _CACHE:
        _CACHE["nc"] = _build_program()
    nc = _CACHE["nc"]

    res = bass_utils.run_bass_kernel_spmd(nc, in_maps, list(range(NCORES)))
    outs = []
    for c in range(NCORES):
        o = res.results[c]["out"]                      # [NG, 128, GH*P]
        o = o.reshape(NG, 2, R, G // 2, P).transpose(0, 3, 1, 2, 4)
        outs.append(o.reshape(L, BLOC, R, P))
    return np.concatenate(outs, axis=1)



# revision 4
# speedup vs baseline: 1.7047x; 1.7047x over previous
"""DETR scene-graph predicate head on 8 Trainium2 NeuronCores.

Math: logits[l,b,r,:] = concat(hs[l,b,q_sub], hs[l,b,q_obj]) @ W_pred.T + b_pred
where q_sub/q_obj come from (tgt_perm inverse, relationships, src_indices) —
pure integer index math, done on host.

Key factorization: split W_pred [P, 2D] into W_sub|W_obj [P, D] halves and
compute, per (layer, image) block, U = matched @ W_sub.T and V = matched @
W_obj.T over the M=64 *matched* queries (matched = hs[l,b,src_indices[l,b]]).
Then logits[r] = U[pos_sub[r]] + V[pos_obj[r]] + b — a cheap index-select the
host applies while unsharding. This cuts device HBM traffic 3x vs shipping
hs+one-hots (each matched row is read once, not once per relation end) and
turns the whole kernel into one W-stationary GEMM.

Device layout (batch axis sharded 8 ways; L*B/8 = 192 blocks/core):
  - Host sends matched.T per block as two 128-row chunks [2, 128, 64] bf16,
    packed 8 blocks/group side-by-side -> rhs tiles [128, 512], two groups per
    DMA [128, 2048] (4 KB/partition lines keep all 16 SDMA engines busy on the
    gpsimd SWDGE queue).
  - Per group: 2 accumulating matmuls with stationary W chunks [128, 102]
    (102 = P*2 outputs) produce UV.T [102, 8*64] f32 in one psum bank:
    psum[:, j*64:(j+1)*64] = (matched_j @ [W_sub.T | W_obj.T]).T.
  - One f32->bf16 cast copy per group (DVE/ACT alternating), one store per
    group on the scalar queue. 48 matmuls total vs 1171 in the one-hot design.

hs and W_pred are bf16 on-chip (psum accumulates f32); U/V return as bf16,
host adds them in f32 — ~3e-3 relative error vs the f32 reference.
"""

import sys

import numpy as np

L, B, Q1, D = 6, 256, 101, 256
M, R, P = 64, 64, 51
NCORES = 8
BLOC = B // NCORES          # images per core
NB = L * BLOC               # (layer, image) blocks per core
G = 8                       # blocks per psum-bank group
NG = NB // G                # groups per core
GPD = 2                     # groups per input DMA
NDMA = NG // GPD            # input DMAs per core
P2 = 2 * P                  # stacked U|V output features

_CACHE = {}


def _build_program():
    import concourse.bacc as bacc
    import concourse.mybir as mybir
    import concourse.tile as tile
    from contextlib import ExitStack

    f32 = mybir.dt.float32
    bf16 = mybir.dt.bfloat16
    nc = bacc.Bacc("TRN2", target_bir_lowering=False, debug=False)

    mt = nc.dram_tensor("mt", [NDMA, 128, GPD * 2 * G * M], bf16,
                        kind="ExternalInput").ap()
    wt = nc.dram_tensor("wt", [128, 2 * P2], bf16, kind="ExternalInput").ap()
    out = nc.dram_tensor("out", [NG, P2, G * M], bf16,
                         kind="ExternalOutput").ap()

    with tile.TileContext(nc) as tc, ExitStack() as ctx:
        const = ctx.enter_context(tc.tile_pool(name="const", bufs=1))
        inp = ctx.enter_context(tc.tile_pool(name="inp", bufs=4))
        outp = ctx.enter_context(tc.tile_pool(name="outp", bufs=4))
        ps = ctx.enter_context(tc.tile_pool(name="ps", bufs=6, space="PSUM"))

        wt_t = const.tile([128, 2 * P2], bf16)
        nc.sync.dma_start(out=wt_t[:], in_=wt[:])

        GW = G * M          # cols per group-chunk rhs tile (512)
        for t in range(NDMA):
            in_t = inp.tile([128, GPD * 2 * GW], bf16, tag="in")
            nc.gpsimd.dma_start(out=in_t[:], in_=mt[t])
            for h in range(GPD):
                g = GPD * t + h
                pg = ps.tile([P2, GW], f32, tag="pg")
                # group h cols: [chunk0 [128, 512] | chunk1 [128, 512]]
                rhs0 = in_t[:, (2 * h) * GW:(2 * h + 1) * GW]
                rhs1 = in_t[:, (2 * h + 1) * GW:(2 * h + 2) * GW]
                # out cols j*64:(j+1)*64 = UV.T of block G*g+j, accumulated
                # over the two 128-row d-chunks of matched.T
                nc.tensor.matmul(out=pg[:], lhsT=wt_t[:, 0:P2],
                                 rhs=rhs0, start=True, stop=False)
                nc.tensor.matmul(out=pg[:], lhsT=wt_t[:, P2:2 * P2],
                                 rhs=rhs1, start=False, stop=True)
                o_t = outp.tile([P2, GW], bf16, tag="o")
                if g % 2 == 0:
                    nc.vector.tensor_copy(out=o_t[:], in_=pg[:])
                else:
                    nc.scalar.copy(out=o_t[:], in_=pg[:])
                nc.scalar.dma_start(out=out[g], in_=o_t[:])

    nc.compile()
    return nc


def _host_indices(src_indices, tgt_perm, relationships):
    """pos_sub, pos_obj: [L, B, R] — position in the matched list per
    relation end (the reference then maps pos -> query via src_indices)."""
    tgt = np.asarray(tgt_perm, dtype=np.int64)
    rel = np.asarray(relationships, dtype=np.int64)

    # lookup[l, b, tgt[l, b, k]] = k
    lookup = np.empty((L, B, M), dtype=np.int64)
    li = np.arange(L)[:, None, None]
    bi = np.arange(B)[None, :, None]
    lookup[li, bi, tgt] = np.broadcast_to(np.arange(M), (L, B, M))

    sub_t = np.broadcast_to(rel[None, :, :, 0], (L, B, R))
    obj_t = np.broadcast_to(rel[None, :, :, 1], (L, B, R))
    pos_sub = np.take_along_axis(lookup, sub_t, axis=2)
    pos_obj = np.take_along_axis(lookup, obj_t, axis=2)
    return pos_sub, pos_obj


def _host_prepare(hs, src_indices, tgt_perm, relationships, W_pred, b_pred):
    """Build per-core input maps (matched rows, transposed + group-packed)."""
    import ml_dtypes
    bf16 = ml_dtypes.bfloat16

    hs = np.asarray(hs, dtype=np.float32)
    src = np.asarray(src_indices, dtype=np.int64)
    W = np.asarray(W_pred, dtype=np.float32)

    # matched rows: hs[l, b, src[l, b, k], :] -> [L, B, M, D]
    matched = np.take_along_axis(hs, src[..., None], axis=2).astype(bf16)

    # W chunks: wt[:, 0:P2] = Wcat[0:128, :], wt[:, P2:] = Wcat[128:256, :]
    # where Wcat [2D? no: D x P2] hmm — Wcat[d, p] = W_sub.T | W_obj.T
    Wcat = np.concatenate([W[:, :D].T, W[:, D:].T], axis=1)    # [D, P2]
    wt_packed = np.ascontiguousarray(
        Wcat.reshape(2, 128, P2).transpose(1, 0, 2).reshape(128, 2 * P2)
    ).astype(bf16)

    in_maps = []
    for c in range(NCORES):
        sl = slice(c * BLOC, (c + 1) * BLOC)
        # [L, BLOC, M, D] -> matched.T chunks [NB, 2, 128, M]
        mt_core = (matched[:, sl].transpose(0, 1, 3, 2)
                   .reshape(NB, 2, 128, M))
        # group-pack: [NG, G, 2, 128, M] -> [NG, 128, 2, G, M]:
        # per group, cols = chunk-major then block-major
        mt_core = mt_core.reshape(NG, G, 2, 128, M).transpose(0, 3, 2, 1, 4)
        # DMA-pack GPD groups per transfer: [NDMA, GPD, 128, 2*G*M]
        mt_core = (mt_core.reshape(NDMA, GPD, 128, 2 * G * M)
                   .transpose(0, 2, 1, 3))
        mt_core = np.ascontiguousarray(
            mt_core.reshape(NDMA, 128, GPD * 2 * G * M))
        in_maps.append({"mt": mt_core, "wt": wt_packed})
    return in_maps


def kernel(hs, src_indices, tgt_perm, relationships, W_pred, b_pred):
    if "concourse" not in sys.modules:
        try:
            import concourse  # noqa: F401
        except ImportError:
            sys.path.insert(0, "/opt/trn_rl_repo")
    from concourse import bass_utils

    in_maps = _host_prepare(hs, src_indices, tgt_perm, relationships,
                            W_pred, b_pred)
    if "nc" not in _CACHE:
        _CACHE["nc"] = _build_program()
    nc = _CACHE["nc"]

    res = bass_utils.run_bass_kernel_spmd(nc, in_maps, list(range(NCORES)))

    # reassemble U, V: out [NG, P2, G*M] -> per block [M, P2] = [U | V]
    uv_cores = []
    for c in range(NCORES):
        o = np.asarray(res.results[c]["out"], dtype=np.float32)
        # [NG, P2, G, M] -> [NG, G, M, P2] -> [L, BLOC, M, P2]
        o = (o.reshape(NG, P2, G, M).transpose(0, 2, 3, 1)
             .reshape(L, BLOC, M, P2))
        uv_cores.append(o)
    uv = np.concatenate(uv_cores, axis=1)                      # [L, B, M, P2]

    pos_sub, pos_obj = _host_indices(src_indices, tgt_perm, relationships)
    U = uv[..., :P]                                            # [L, B, M, P]
    V = uv[..., P:]
    b = np.asarray(b_pred, dtype=np.float32)
    logits = (np.take_along_axis(U, pos_sub[..., None], axis=2)
              + np.take_along_axis(V, pos_obj[..., None], axis=2) + b)
    return np.ascontiguousarray(logits, dtype=np.float32)


# revision 7
# speedup vs baseline: 1.8974x; 1.1130x over previous
"""DETR scene-graph predicate head on 8 Trainium2 NeuronCores.

Math: logits[l,b,r,:] = concat(hs[l,b,q_sub], hs[l,b,q_obj]) @ W_pred.T + b_pred
where q_sub/q_obj come from (tgt_perm inverse, relationships, src_indices) —
pure integer index math, done on host.

Key factorization: split W_pred [P, 2D] into W_sub|W_obj [P, D] halves and
compute, per (layer, image) block, U = matched @ W_sub.T and V = matched @
W_obj.T over the M=64 *matched* queries (matched = hs[l,b,src_indices[l,b]]).
Then logits[r] = U[pos_sub[r]] + V[pos_obj[r]] + b — a cheap index-select the
host applies while unsharding. This cuts device HBM traffic 3x vs shipping
hs+one-hots (each matched row is read once, not once per relation end) and
turns the whole kernel into one W-stationary GEMM.

Device layout (batch axis sharded 8 ways; L*B/8 = 192 blocks/core):
  - Host sends matched.T per block as two 128-row chunks [2, 128, 64] bf16,
    packed 8 blocks/group side-by-side -> rhs tiles [128, 512], two groups per
    DMA [128, 2048] (4 KB/partition lines keep all 16 SDMA engines busy on the
    gpsimd SWDGE queue).
  - Per group: 2 accumulating matmuls with stationary W chunks [128, 102]
    (102 = P*2 outputs) produce UV.T [102, 8*64] f32 in one psum bank:
    psum[:, j*64:(j+1)*64] = (matched_j @ [W_sub.T | W_obj.T]).T.
  - One f32->bf16 cast copy per group (DVE/ACT alternating), one store per
    group on the scalar queue. 48 matmuls total vs 1171 in the one-hot design.

hs and W_pred are bf16 on-chip (psum accumulates f32); U/V return as bf16,
host adds them in f32 — ~3e-3 relative error vs the f32 reference.
"""

import sys

import numpy as np

L, B, Q1, D = 6, 256, 101, 256
M, R, P = 64, 64, 51
NCORES = 8
BLOC = B // NCORES          # images per core
NB = L * BLOC               # (layer, image) blocks per core
G = 8                       # blocks per psum-bank group
NG = NB // G                # groups per core
GPD = 2                     # groups per input DMA
NDMA = NG // GPD            # input DMAs per core
GPO = 4                     # groups per output DMA
NODMA = NG // GPO           # output DMAs per core
P2 = 2 * P                  # stacked U|V output features

_CACHE = {}


def _build_program():
    import concourse.bacc as bacc
    import concourse.mybir as mybir
    import concourse.tile as tile
    from contextlib import ExitStack

    f32 = mybir.dt.float32
    bf16 = mybir.dt.bfloat16
    nc = bacc.Bacc("TRN2", target_bir_lowering=False, debug=False)

    mt = nc.dram_tensor("mt", [NDMA, 128, GPD * 2 * G * M], bf16,
                        kind="ExternalInput").ap()
    wt = nc.dram_tensor("wt", [128, 2 * P2], bf16, kind="ExternalInput").ap()
    out = nc.dram_tensor("out", [NODMA, P2, GPO * G * M], bf16,
                         kind="ExternalOutput").ap()

    with tile.TileContext(nc) as tc, ExitStack() as ctx:
        const = ctx.enter_context(tc.tile_pool(name="const", bufs=1))
        inp = ctx.enter_context(tc.tile_pool(name="inp", bufs=4))
        outp = ctx.enter_context(tc.tile_pool(name="outp", bufs=3))
        ps = ctx.enter_context(tc.tile_pool(name="ps", bufs=6, space="PSUM"))

        # const load on the 16-engine gpsimd SWDGE queue: the sync queue
        # runs at single-engine rate (~9us for 128 small lines) and every
        # matmul would wait on it
        wt_t = const.tile([128, 2 * P2], bf16)
        nc.gpsimd.dma_start(out=wt_t[:], in_=wt[:])

        GW = G * M          # cols per group-chunk rhs tile (512)
        for t in range(NDMA):
            in_t = inp.tile([128, GPD * 2 * GW], bf16, tag="in")
            nc.gpsimd.dma_start(out=in_t[:], in_=mt[t])
            for h in range(GPD):
                g = GPD * t + h
                if g % GPO == 0:
                    o_t = outp.tile([P2, GPO * GW], bf16, tag="o")
                pg = ps.tile([P2, GW], f32, tag="pg")
                # group h cols: [chunk0 [128, 512] | chunk1 [128, 512]]
                rhs0 = in_t[:, (2 * h) * GW:(2 * h + 1) * GW]
                rhs1 = in_t[:, (2 * h + 1) * GW:(2 * h + 2) * GW]
                # out cols j*64:(j+1)*64 = UV.T of block G*g+j, accumulated
                # over the two 128-row d-chunks of matched.T
                nc.tensor.matmul(out=pg[:], lhsT=wt_t[:, 0:P2],
                                 rhs=rhs0, start=True, stop=False)
                nc.tensor.matmul(out=pg[:], lhsT=wt_t[:, P2:2 * P2],
                                 rhs=rhs1, start=False, stop=True)
                oc = o_t[:, (g % GPO) * GW:(g % GPO + 1) * GW]
                if g % 2 == 0:
                    nc.vector.tensor_copy(out=oc, in_=pg[:])
                else:
                    nc.scalar.copy(out=oc, in_=pg[:])
                if g % GPO == GPO - 1:
                    nc.scalar.dma_start(out=out[g // GPO], in_=o_t[:])

    nc.compile()
    return nc


def _host_indices(src_indices, tgt_perm, relationships):
    """pos_sub, pos_obj: [L, B, R] — position in the matched list per
    relation end (the reference then maps pos -> query via src_indices)."""
    tgt = np.asarray(tgt_perm, dtype=np.int64)
    rel = np.asarray(relationships, dtype=np.int64)

    # lookup[l, b, tgt[l, b, k]] = k
    lookup = np.empty((L, B, M), dtype=np.int64)
    li = np.arange(L)[:, None, None]
    bi = np.arange(B)[None, :, None]
    lookup[li, bi, tgt] = np.broadcast_to(np.arange(M), (L, B, M))

    sub_t = np.broadcast_to(rel[None, :, :, 0], (L, B, R))
    obj_t = np.broadcast_to(rel[None, :, :, 1], (L, B, R))
    pos_sub = np.take_along_axis(lookup, sub_t, axis=2)
    pos_obj = np.take_along_axis(lookup, obj_t, axis=2)
    return pos_sub, pos_obj


def _host_prepare(hs, src_indices, tgt_perm, relationships, W_pred, b_pred):
    """Build per-core input maps (matched rows, transposed + group-packed)."""
    import ml_dtypes
    bf16 = ml_dtypes.bfloat16

    hs = np.asarray(hs, dtype=np.float32)
    src = np.asarray(src_indices, dtype=np.int64)
    W = np.asarray(W_pred, dtype=np.float32)

    # matched rows: hs[l, b, src[l, b, k], :] -> [L, B, M, D]
    matched = np.take_along_axis(hs, src[..., None], axis=2).astype(bf16)

    # W chunks: wt[:, 0:P2] = Wcat[0:128, :], wt[:, P2:] = Wcat[128:256, :]
    # where Wcat [2D? no: D x P2] hmm — Wcat[d, p] = W_sub.T | W_obj.T
    Wcat = np.concatenate([W[:, :D].T, W[:, D:].T], axis=1)    # [D, P2]
    wt_packed = np.ascontiguousarray(
        Wcat.reshape(2, 128, P2).transpose(1, 0, 2).reshape(128, 2 * P2)
    ).astype(bf16)

    in_maps = []
    for c in range(NCORES):
        sl = slice(c * BLOC, (c + 1) * BLOC)
        # [L, BLOC, M, D] -> matched.T chunks [NB, 2, 128, M]
        mt_core = (matched[:, sl].transpose(0, 1, 3, 2)
                   .reshape(NB, 2, 128, M))
        # group-pack: [NG, G, 2, 128, M] -> [NG, 128, 2, G, M]:
        # per group, cols = chunk-major then block-major
        mt_core = mt_core.reshape(NG, G, 2, 128, M).transpose(0, 3, 2, 1, 4)
        # DMA-pack GPD groups per transfer: [NDMA, GPD, 128, 2*G*M]
        mt_core = (mt_core.reshape(NDMA, GPD, 128, 2 * G * M)
                   .transpose(0, 2, 1, 3))
        mt_core = np.ascontiguousarray(
            mt_core.reshape(NDMA, 128, GPD * 2 * G * M))
        in_maps.append({"mt": mt_core, "wt": wt_packed})
    return in_maps


def kernel(hs, src_indices, tgt_perm, relationships, W_pred, b_pred):
    if "concourse" not in sys.modules:
        try:
            import concourse  # noqa: F401
        except ImportError:
            sys.path.insert(0, "/opt/trn_rl_repo")
    from concourse import bass_utils

    in_maps = _host_prepare(hs, src_indices, tgt_perm, relationships,
                            W_pred, b_pred)
    if "nc" not in _CACHE:
        _CACHE["nc"] = _build_program()
    nc = _CACHE["nc"]

    res = bass_utils.run_bass_kernel_spmd(nc, in_maps, list(range(NCORES)))

    # reassemble U, V: out [NG, P2, G*M] -> per block [M, P2] = [U | V]
    uv_cores = []
    for c in range(NCORES):
        o = np.asarray(res.results[c]["out"], dtype=np.float32)
        # [NODMA, P2, GPO*G*M] -> [NG, P2, G*M] -> [L, BLOC, M, P2]
        o = o.reshape(NODMA, P2, GPO, G * M).transpose(0, 2, 1, 3)
        o = (o.reshape(NG, P2, G, M).transpose(0, 2, 3, 1)
             .reshape(L, BLOC, M, P2))
        uv_cores.append(o)
    uv = np.concatenate(uv_cores, axis=1)                      # [L, B, M, P2]

    pos_sub, pos_obj = _host_indices(src_indices, tgt_perm, relationships)
    U = uv[..., :P]                                            # [L, B, M, P]
    V = uv[..., P:]
    b = np.asarray(b_pred, dtype=np.float32)
    logits = (np.take_along_axis(U, pos_sub[..., None], axis=2)
              + np.take_along_axis(V, pos_obj[..., None], axis=2) + b)
    return np.ascontiguousarray(logits, dtype=np.float32)


# revision 9
# speedup vs baseline: 1.9095x; 1.0064x over previous
"""DETR scene-graph predicate head on 8 Trainium2 NeuronCores.

Math: logits[l,b,r,:] = concat(hs[l,b,q_sub], hs[l,b,q_obj]) @ W_pred.T + b_pred
where q_sub/q_obj come from (tgt_perm inverse, relationships, src_indices) —
pure integer index math, done on host.

Key factorization: split W_pred [P, 2D] into W_sub|W_obj [P, D] halves and
compute, per (layer, image) block, U = matched @ W_sub.T and V = matched @
W_obj.T over the M=64 *matched* queries (matched = hs[l,b,src_indices[l,b]]).
Then logits[r] = U[pos_sub[r]] + V[pos_obj[r]] + b — a cheap index-select the
host applies while unsharding. This cuts device HBM traffic 3x vs shipping
hs+one-hots (each matched row is read once, not once per relation end) and
turns the whole kernel into one W-stationary GEMM.

Device layout (batch axis sharded 8 ways; L*B/8 = 192 blocks/core):
  - Host sends matched.T per block as two 128-row chunks [2, 128, 64] bf16,
    packed 8 blocks/group side-by-side -> rhs tiles [128, 512], two groups per
    DMA [128, 2048] (4 KB/partition lines keep all 16 SDMA engines busy on the
    gpsimd SWDGE queue).
  - Per group: 2 accumulating matmuls with stationary W chunks [128, 102]
    (102 = P*2 outputs) produce UV.T [102, 8*64] f32 in one psum bank:
    psum[:, j*64:(j+1)*64] = (matched_j @ [W_sub.T | W_obj.T]).T.
  - One f32->bf16 cast copy per group (DVE/ACT alternating), one store per
    group on the scalar queue. 48 matmuls total vs 1171 in the one-hot design.

hs and W_pred are bf16 on-chip (psum accumulates f32); U/V return as bf16,
host adds them in f32 — ~3e-3 relative error vs the f32 reference.
"""

import sys

import numpy as np

L, B, Q1, D = 6, 256, 101, 256
M, R, P = 64, 64, 51
NCORES = 8
BLOC = B // NCORES          # images per core
NB = L * BLOC               # (layer, image) blocks per core
G = 8                       # blocks per psum-bank group
NG = NB // G                # groups per core
GPD = 4                     # groups per input DMA
NDMA = NG // GPD            # input DMAs per core
GPO = 4                     # groups per output DMA
NODMA = NG // GPO           # output DMAs per core
P2 = 2 * P                  # stacked U|V output features

_CACHE = {}


def _build_program():
    import concourse.bacc as bacc
    import concourse.mybir as mybir
    import concourse.tile as tile
    from contextlib import ExitStack

    f32 = mybir.dt.float32
    bf16 = mybir.dt.bfloat16
    nc = bacc.Bacc("TRN2", target_bir_lowering=False, debug=False)

    mt = nc.dram_tensor("mt", [NDMA, 128, GPD * 2 * G * M], bf16,
                        kind="ExternalInput").ap()
    wt = nc.dram_tensor("wt", [128, 2 * P2], bf16, kind="ExternalInput").ap()
    out = nc.dram_tensor("out", [NODMA, P2, GPO * G * M], bf16,
                         kind="ExternalOutput").ap()

    with tile.TileContext(nc) as tc, ExitStack() as ctx:
        const = ctx.enter_context(tc.tile_pool(name="const", bufs=1))
        inp = ctx.enter_context(tc.tile_pool(name="inp", bufs=4))
        outp = ctx.enter_context(tc.tile_pool(name="outp", bufs=3))
        ps = ctx.enter_context(tc.tile_pool(name="ps", bufs=6, space="PSUM"))

        # const load on the scalar queue (idle at start): keeps wt's 128
        # small lines off both the single-engine sync queue (~9us serial)
        # and the gpsimd input queue (where they'd delay mt[0])
        wt_t = const.tile([128, 2 * P2], bf16)
        nc.scalar.dma_start(out=wt_t[:], in_=wt[:])

        GW = G * M          # cols per group-chunk rhs tile (512)
        for t in range(NDMA):
            in_t = inp.tile([128, GPD * 2 * GW], bf16, tag="in")
            nc.gpsimd.dma_start(out=in_t[:], in_=mt[t])
            for h in range(GPD):
                g = GPD * t + h
                if g % GPO == 0:
                    o_t = outp.tile([P2, GPO * GW], bf16, tag="o")
                pg = ps.tile([P2, GW], f32, tag="pg")
                # group h cols: [chunk0 [128, 512] | chunk1 [128, 512]]
                rhs0 = in_t[:, (2 * h) * GW:(2 * h + 1) * GW]
                rhs1 = in_t[:, (2 * h + 1) * GW:(2 * h + 2) * GW]
                # out cols j*64:(j+1)*64 = UV.T of block G*g+j, accumulated
                # over the two 128-row d-chunks of matched.T
                nc.tensor.matmul(out=pg[:], lhsT=wt_t[:, 0:P2],
                                 rhs=rhs0, start=True, stop=False)
                nc.tensor.matmul(out=pg[:], lhsT=wt_t[:, P2:2 * P2],
                                 rhs=rhs1, start=False, stop=True)
                oc = o_t[:, (g % GPO) * GW:(g % GPO + 1) * GW]
                if g % 2 == 0:
                    nc.vector.tensor_copy(out=oc, in_=pg[:])
                else:
                    nc.scalar.copy(out=oc, in_=pg[:])
                if g % GPO == GPO - 1:
                    nc.scalar.dma_start(out=out[g // GPO], in_=o_t[:])

    nc.compile()
    return nc


def _host_indices(src_indices, tgt_perm, relationships):
    """pos_sub, pos_obj: [L, B, R] — position in the matched list per
    relation end (the reference then maps pos -> query via src_indices)."""
    tgt = np.asarray(tgt_perm, dtype=np.int64)
    rel = np.asarray(relationships, dtype=np.int64)

    # lookup[l, b, tgt[l, b, k]] = k
    lookup = np.empty((L, B, M), dtype=np.int64)
    li = np.arange(L)[:, None, None]
    bi = np.arange(B)[None, :, None]
    lookup[li, bi, tgt] = np.broadcast_to(np.arange(M), (L, B, M))

    sub_t = np.broadcast_to(rel[None, :, :, 0], (L, B, R))
    obj_t = np.broadcast_to(rel[None, :, :, 1], (L, B, R))
    pos_sub = np.take_along_axis(lookup, sub_t, axis=2)
    pos_obj = np.take_along_axis(lookup, obj_t, axis=2)
    return pos_sub, pos_obj


def _host_prepare(hs, src_indices, tgt_perm, relationships, W_pred, b_pred):
    """Build per-core input maps (matched rows, transposed + group-packed)."""
    import ml_dtypes
    bf16 = ml_dtypes.bfloat16

    hs = np.asarray(hs, dtype=np.float32)
    src = np.asarray(src_indices, dtype=np.int64)
    W = np.asarray(W_pred, dtype=np.float32)

    # matched rows: hs[l, b, src[l, b, k], :] -> [L, B, M, D]
    matched = np.take_along_axis(hs, src[..., None], axis=2).astype(bf16)

    # W chunks: wt[:, 0:P2] = Wcat[0:128, :], wt[:, P2:] = Wcat[128:256, :]
    # where Wcat [2D? no: D x P2] hmm — Wcat[d, p] = W_sub.T | W_obj.T
    Wcat = np.concatenate([W[:, :D].T, W[:, D:].T], axis=1)    # [D, P2]
    wt_packed = np.ascontiguousarray(
        Wcat.reshape(2, 128, P2).transpose(1, 0, 2).reshape(128, 2 * P2)
    ).astype(bf16)

    in_maps = []
    for c in range(NCORES):
        sl = slice(c * BLOC, (c + 1) * BLOC)
        # [L, BLOC, M, D] -> matched.T chunks [NB, 2, 128, M]
        mt_core = (matched[:, sl].transpose(0, 1, 3, 2)
                   .reshape(NB, 2, 128, M))
        # group-pack: [NG, G, 2, 128, M] -> [NG, 128, 2, G, M]:
        # per group, cols = chunk-major then block-major
        mt_core = mt_core.reshape(NG, G, 2, 128, M).transpose(0, 3, 2, 1, 4)
        # DMA-pack GPD groups per transfer: [NDMA, GPD, 128, 2*G*M]
        mt_core = (mt_core.reshape(NDMA, GPD, 128, 2 * G * M)
                   .transpose(0, 2, 1, 3))
        mt_core = np.ascontiguousarray(
            mt_core.reshape(NDMA, 128, GPD * 2 * G * M))
        in_maps.append({"mt": mt_core, "wt": wt_packed})
    return in_maps


def kernel(hs, src_indices, tgt_perm, relationships, W_pred, b_pred):
    if "concourse" not in sys.modules:
        try:
            import concourse  # noqa: F401
        except ImportError:
            sys.path.insert(0, "/opt/trn_rl_repo")
    from concourse import bass_utils

    in_maps = _host_prepare(hs, src_indices, tgt_perm, relationships,
                            W_pred, b_pred)
    if "nc" not in _CACHE:
        _CACHE["nc"] = _build_program()
    nc = _CACHE["nc"]

    res = bass_utils.run_bass_kernel_spmd(nc, in_maps, list(range(NCORES)))

    # reassemble U, V: out [NG, P2, G*M] -> per block [M, P2] = [U | V]
    uv_cores = []
    for c in range(NCORES):
        o = np.asarray(res.results[c]["out"], dtype=np.float32)
        # [NODMA, P2, GPO*G*M] -> [NG, P2, G*M] -> [L, BLOC, M, P2]
        o = o.reshape(NODMA, P2, GPO, G * M).transpose(0, 2, 1, 3)
        o = (o.reshape(NG, P2, G, M).transpose(0, 2, 3, 1)
             .reshape(L, BLOC, M, P2))
        uv_cores.append(o)
    uv = np.concatenate(uv_cores, axis=1)                      # [L, B, M, P2]

    pos_sub, pos_obj = _host_indices(src_indices, tgt_perm, relationships)
    U = uv[..., :P]                                            # [L, B, M, P]
    V = uv[..., P:]
    b = np.asarray(b_pred, dtype=np.float32)
    logits = (np.take_along_axis(U, pos_sub[..., None], axis=2)
              + np.take_along_axis(V, pos_obj[..., None], axis=2) + b)
    return np.ascontiguousarray(logits, dtype=np.float32)


# revision 14
# speedup vs baseline: 2.1121x; 1.1061x over previous
"""DETR scene-graph predicate head on 8 Trainium2 NeuronCores.

Math: logits[l,b,r,:] = concat(hs[l,b,q_sub], hs[l,b,q_obj]) @ W_pred.T + b_pred
where q_sub/q_obj come from (tgt_perm inverse, relationships, src_indices) —
pure integer index math, done on host.

Key factorization: split W_pred [P, 2D] into W_sub|W_obj [P, D] halves and
compute, per (layer, image) block, U = matched @ W_sub.T and V = matched @
W_obj.T over the M=64 *matched* queries (matched = hs[l,b,src_indices[l,b]]).
Then logits[r] = U[pos_sub[r]] + V[pos_obj[r]] + b — a cheap index-select the
host applies while unsharding. This cuts device HBM traffic 3x vs shipping
hs+one-hots (each matched row is read once, not once per relation end) and
turns the whole kernel into one W-stationary GEMM.

Device layout (batch axis sharded 8 ways; L*B/8 = 192 blocks/core):
  - Host sends matched.T per block as two 128-row chunks [2, 128, 64] bf16,
    packed 8 blocks/group side-by-side -> rhs tiles [128, 512], two groups per
    DMA [128, 2048] (4 KB/partition lines keep all 16 SDMA engines busy on the
    gpsimd SWDGE queue).
  - Per group: 2 accumulating matmuls with stationary W chunks [128, 102]
    (102 = P*2 outputs) produce UV.T [102, 8*64] f32 in one psum bank:
    psum[:, j*64:(j+1)*64] = (matched_j @ [W_sub.T | W_obj.T]).T.
  - One f32->bf16 cast copy per group (DVE/ACT alternating), one store per
    group on the scalar queue. 48 matmuls total vs 1171 in the one-hot design.

hs and W_pred are bf16 on-chip (psum accumulates f32); U/V return as bf16,
host adds them in f32 — ~3e-3 relative error vs the f32 reference.
"""

import sys

import numpy as np

L, B, Q1, D = 6, 256, 101, 256
M, R, P = 64, 64, 51
NCORES = 8
BLOC = B // NCORES          # images per core
NB = L * BLOC               # (layer, image) blocks per core
G = 8                       # blocks per psum-bank group
NG = NB // G                # groups per core
ISPLIT = (0, 2, 6, 12, 18, 24)   # input DMA boundaries, in groups
GPO = 4                     # groups per output DMA
NODMA = NG // GPO           # output DMAs per core
P2 = 2 * P                  # stacked U|V output features

_CACHE = {}


def _build_program():
    import concourse.bacc as bacc
    import concourse.mybir as mybir
    import concourse.tile as tile
    from contextlib import ExitStack

    f32 = mybir.dt.float32
    bf16 = mybir.dt.bfloat16
    nc = bacc.Bacc("TRN2", target_bir_lowering=False, debug=False)

    GW = G * M              # cols per group-chunk rhs tile (512)
    mt = nc.dram_tensor("mt", [128, NG * 2 * GW], bf16,
                        kind="ExternalInput").ap()
    wt = nc.dram_tensor("wt", [128, 2 * P2], bf16, kind="ExternalInput").ap()
    out = nc.dram_tensor("out", [P2, NG * GW], bf16,
                         kind="ExternalOutput").ap()

    with tile.TileContext(nc) as tc, ExitStack() as ctx:
        # static arenas (everything fits in SBUF): no pool rotation, so no
        # WAR semaphores; one tile per input DMA keeps dep tracking exact
        sb = ctx.enter_context(tc.tile_pool(name="sb", bufs=1))
        ps = ctx.enter_context(tc.tile_pool(name="ps", bufs=4, space="PSUM"))

        # const load on the scalar queue so it neither rides the
        # single-engine sync queue (~9us serial for 128 small lines) nor
        # delays mt[0] on the gpsimd queue
        wt_t = sb.tile([128, 2 * P2], bf16, tag="wt")
        nc.scalar.dma_start(out=wt_t[:], in_=wt[:])

        in_tiles = []
        for i, (a, b) in enumerate(zip(ISPLIT, ISPLIT[1:])):
            in_t = sb.tile([128, (b - a) * 2 * GW], bf16, tag=f"in{i}")
            nc.gpsimd.dma_start(out=in_t[:],
                                in_=mt[:, a * 2 * GW:b * 2 * GW])
            in_tiles.append(in_t)

        o_t = sb.tile([P2, NG * GW], bf16, tag="o")

        def rhs_of(g):
            i = max(k for k, a in enumerate(ISPLIT[:-1]) if ISPLIT[k] <= g)
            off = (g - ISPLIT[i]) * 2 * GW
            return in_tiles[i][:, off:off + 2 * GW]

        for p in range(NG // 2):        # pairs of groups
            pq = ps.tile([P2, 2 * GW], f32, tag="pq")
            for h in range(2):
                rhs = rhs_of(2 * p + h)
                # out cols j*64:(j+1)*64 = UV.T of block, accumulated over
                # the two 128-row d-chunks of matched.T
                nc.tensor.matmul(out=pq[:, h * GW:(h + 1) * GW],
                                 lhsT=wt_t[:, 0:P2], rhs=rhs[:, 0:GW],
                                 start=True, stop=False)
                nc.tensor.matmul(out=pq[:, h * GW:(h + 1) * GW],
                                 lhsT=wt_t[:, P2:2 * P2], rhs=rhs[:, GW:2 * GW],
                                 start=False, stop=True)
            oc = o_t[:, p * 2 * GW:(p + 1) * 2 * GW]
            if p % 2 == 0:
                nc.vector.tensor_copy(out=oc, in_=pq[:])
            else:
                nc.scalar.copy(out=oc, in_=pq[:])
            if p % 2 == 1:              # one output DMA per 4 groups
                s = (p - 1) * 2 * GW
                nc.scalar.dma_start(out=out[:, s:s + 4 * GW],
                                    in_=o_t[:, s:s + 4 * GW])

    nc.compile()
    return nc


def _host_indices(src_indices, tgt_perm, relationships):
    """pos_sub, pos_obj: [L, B, R] — position in the matched list per
    relation end (the reference then maps pos -> query via src_indices)."""
    tgt = np.asarray(tgt_perm, dtype=np.int64)
    rel = np.asarray(relationships, dtype=np.int64)

    # lookup[l, b, tgt[l, b, k]] = k
    lookup = np.empty((L, B, M), dtype=np.int64)
    li = np.arange(L)[:, None, None]
    bi = np.arange(B)[None, :, None]
    lookup[li, bi, tgt] = np.broadcast_to(np.arange(M), (L, B, M))

    sub_t = np.broadcast_to(rel[None, :, :, 0], (L, B, R))
    obj_t = np.broadcast_to(rel[None, :, :, 1], (L, B, R))
    pos_sub = np.take_along_axis(lookup, sub_t, axis=2)
    pos_obj = np.take_along_axis(lookup, obj_t, axis=2)
    return pos_sub, pos_obj


def _host_prepare(hs, src_indices, tgt_perm, relationships, W_pred, b_pred):
    """Build per-core input maps (matched rows, transposed + group-packed)."""
    import ml_dtypes
    bf16 = ml_dtypes.bfloat16

    hs = np.asarray(hs, dtype=np.float32)
    src = np.asarray(src_indices, dtype=np.int64)
    W = np.asarray(W_pred, dtype=np.float32)

    # matched rows: hs[l, b, src[l, b, k], :] -> [L, B, M, D]
    matched = np.take_along_axis(hs, src[..., None], axis=2).astype(bf16)

    # W chunks: wt[:, 0:P2] = Wcat[0:128, :], wt[:, P2:] = Wcat[128:256, :]
    # where Wcat [2D? no: D x P2] hmm — Wcat[d, p] = W_sub.T | W_obj.T
    Wcat = np.concatenate([W[:, :D].T, W[:, D:].T], axis=1)    # [D, P2]
    wt_packed = np.ascontiguousarray(
        Wcat.reshape(2, 128, P2).transpose(1, 0, 2).reshape(128, 2 * P2)
    ).astype(bf16)

    in_maps = []
    for c in range(NCORES):
        sl = slice(c * BLOC, (c + 1) * BLOC)
        # [L, BLOC, M, D] -> matched.T chunks [NB, 2, 128, M]
        mt_core = (matched[:, sl].transpose(0, 1, 3, 2)
                   .reshape(NB, 2, 128, M))
        # group-pack: [NG, G, 2, 128, M] -> [128, NG, 2, G, M]: flat col
        # order (group, chunk, block, k), partition dim first
        mt_core = mt_core.reshape(NG, G, 2, 128, M).transpose(3, 0, 2, 1, 4)
        mt_core = np.ascontiguousarray(mt_core.reshape(128, NG * 2 * G * M))
        in_maps.append({"mt": mt_core, "wt": wt_packed})
    return in_maps


def kernel(hs, src_indices, tgt_perm, relationships, W_pred, b_pred):
    if "concourse" not in sys.modules:
        try:
            import concourse  # noqa: F401
        except ImportError:
            sys.path.insert(0, "/opt/trn_rl_repo")
    from concourse import bass_utils

    in_maps = _host_prepare(hs, src_indices, tgt_perm, relationships,
                            W_pred, b_pred)
    if "nc" not in _CACHE:
        _CACHE["nc"] = _build_program()
    nc = _CACHE["nc"]

    res = bass_utils.run_bass_kernel_spmd(nc, in_maps, list(range(NCORES)))

    # reassemble U, V: out [NG, P2, G*M] -> per block [M, P2] = [U | V]
    uv_cores = []
    for c in range(NCORES):
        o = np.asarray(res.results[c]["out"], dtype=np.float32)
        # [P2, NG*G*M] cols (g, j, k) -> [L, BLOC, M, P2]
        o = (o.reshape(P2, NG, G, M).transpose(1, 2, 3, 0)
             .reshape(L, BLOC, M, P2))
        uv_cores.append(o)
    uv = np.concatenate(uv_cores, axis=1)                      # [L, B, M, P2]

    pos_sub, pos_obj = _host_indices(src_indices, tgt_perm, relationships)
    U = uv[..., :P]                                            # [L, B, M, P]
    V = uv[..., P:]
    b = np.asarray(b_pred, dtype=np.float32)
    logits = (np.take_along_axis(U, pos_sub[..., None], axis=2)
              + np.take_along_axis(V, pos_obj[..., None], axis=2) + b)
    return np.ascontiguousarray(logits, dtype=np.float32)
